# revision 4
# baseline (speedup 1.0000x reference)
"""Per-pixel dynamic-filter 5x5 convolution (KPN-style) on 8 TRN2 NeuronCores.

Math: out[b,h,w] = sum_{di,dj,c} img[b, h+di-2, w+dj-2, c] * filts[b, h, w, (di*5+dj)*3+c]
Shapes: img [4,512,512,3] f32, filts [4,512,512,75] f32 -> out [4,512,512] f32.

Strategy (pure data parallel, no cross-core comms):
  - 8 shards = (batch b in 0..3) x (H half in 0..1); each core owns a
    [256, 512] output slab.
  - Host prep (per core): pad img spatially, transpose to [h', c, x] fp16;
    transpose filts to [h, kk, w] fp16 so every per-kk plane is w-contiguous.
  - On-chip, per 128-row tile: 5 partition-shifted img copies (one per di,
    DMA'd at row offset di) + 5 one-element-x-shifted copies (ACT) so every
    DVE operand is 4B-aligned -> fp16 tensor_tensor runs in 2x mode.
  - 75 DVE multiplies produce fp16 product planes; the TensorEngine
    accumulates them in fp32 PSUM via identity matmuls (start/stop group).
  - ACT evicts PSUM->SBUF fp32, DMA out.
"""

import sys

sys.path.insert(0, "/opt/trn_rl_repo")

import numpy as np

from concourse import bass, bacc, mybir
from concourse.tile import TileContext
from concourse.bass_utils import run_bass_kernel_spmd

B, H, W, C = 4, 512, 512, 3
K = 5
KK = K * K * C  # 75
N_CORES = 8
HSH = H // 2  # 256 rows per shard
XP = W + 6  # padded x extent: w in [-2, 516) -> x = w+2 in [0, 518)
IMG_FREE = C * XP  # 1554 fp16 elements per padded img row
N_HT = HSH // 128  # 2 h-tiles per shard
KCH = 25  # filts kk-chunk per DMA
N_CH = KK // KCH  # 3

_F16 = mybir.dt.float16
_F32 = mybir.dt.float32

_NC = None


def build_nc():
    """Build the single-core Bass program (identical on all 8 cores)."""
    # Bacc (not plain Bass): its compile() pass splits multi-semaphore waits
    # into EventSemaphore instructions — walrus allows only 1 wait per
    # compute instruction.
    nc = bacc.Bacc("TRN2")
    img_d = nc.declare_dram_parameter("img", [HSH + 4, IMG_FREE], _F16, isOutput=False)
    filts_d = nc.declare_dram_parameter("filts", [HSH, KK * W], _F16, isOutput=False)
    ident_d = nc.declare_dram_parameter("ident", [128, 128], _F16, isOutput=False)
    out_d = nc.declare_dram_parameter("out", [HSH, W], _F32, isOutput=True)

    with TileContext(nc) as tc:
        with (
            tc.tile_pool(name="const", bufs=1) as constp,
            tc.tile_pool(name="imgp", bufs=2) as imgp,
            tc.tile_pool(name="filtp", bufs=3) as filtp,
            tc.tile_pool(name="prodp", bufs=4) as prodp,
            tc.tile_pool(name="outp", bufs=2) as outp,
            tc.tile_pool(name="psump", bufs=2, space="PSUM") as psump,
        ):
            id_t = constp.tile([128, 128], _F16)
            nc.sync.dma_start(out=id_t[:], in_=ident_d[:])

            for ht in range(N_HT):
                r0 = ht * 128
                imgs = {}
                for di in range(K):
                    t0 = imgp.tile([128, IMG_FREE], _F16, tag=f"img0_{di}")
                    nc.sync.dma_start(out=t0[:], in_=img_d[r0 + di : r0 + di + 128, :])
                    # x-shifted-by-one copy keeps odd-dj operands 4B-aligned
                    t1 = imgp.tile([128, IMG_FREE], _F16, tag=f"img1_{di}")
                    nc.scalar.copy(out=t1[:, 0 : IMG_FREE - 1], in_=t0[:, 1:IMG_FREE])
                    imgs[(di, 0)] = t0
                    imgs[(di, 1)] = t1

                psum_t = psump.tile([128, W], _F32)
                for ch in range(N_CH):
                    ft = filtp.tile([128, KCH * W], _F16, tag="ft")
                    nc.sync.dma_start(
                        out=ft[:],
                        in_=filts_d[r0 : r0 + 128, ch * KCH * W : (ch + 1) * KCH * W],
                    )
                    for i in range(KCH):
                        kk = ch * KCH + i
                        di = kk // (K * C)
                        dj = (kk // C) % K
                        c = kk % C
                        q = dj & 1
                        x0 = c * XP + (dj - q)
                        p_t = prodp.tile([128, W], _F16, tag="pt")
                        nc.vector.tensor_tensor(
                            p_t[:],
                            imgs[(di, q)][:, x0 : x0 + W],
                            ft[:, i * W : (i + 1) * W],
                            mybir.AluOpType.mult,
                        )
                        nc.tensor.matmul(
                            psum_t[:],
                            id_t[:],
                            p_t[:],
                            start=(kk == 0),
                            stop=(kk == KK - 1),
                        )

                o_t = outp.tile([128, W], _F32, tag="ot")
                nc.scalar.copy(out=o_t[:], in_=psum_t[:])
                nc.sync.dma_start(out=out_d[r0 : r0 + 128, :], in_=o_t[:])

    nc.compile()
    return nc


def get_nc():
    global _NC
    if _NC is None:
        _NC = build_nc()
    return _NC


def prepare_in_maps(img_stack: np.ndarray, filts: np.ndarray):
    """Shard + reformat FULL fp32 inputs into per-core fp16 input maps."""
    ident = np.eye(128, dtype=np.float16)
    in_maps = []
    for core in range(N_CORES):
        b, hh = divmod(core, 2)
        h0 = hh * HSH
        # img: pad h by 2 each side, w by 2 left / 4 right -> [516, 518, 3]
        padded = np.pad(img_stack[b], ((2, 2), (2, XP - W - 2), (0, 0)))
        shard = padded[h0 : h0 + HSH + 4]  # rows h0-2 .. h0+258
        img_p = (
            np.ascontiguousarray(shard.transpose(0, 2, 1))
            .astype(np.float16)
            .reshape(HSH + 4, IMG_FREE)
        )
        # filts: [256, 512, 75] -> [256, 75, 512]
        filts_p = (
            np.ascontiguousarray(filts[b, h0 : h0 + HSH].transpose(0, 2, 1))
            .astype(np.float16)
            .reshape(HSH, KK * W)
        )
        in_maps.append({"img": img_p, "filts": filts_p, "ident": ident})
    return in_maps


def assemble_out(results) -> np.ndarray:
    out = np.empty((B, H, W), dtype=np.float32)
    for core in range(N_CORES):
        b, hh = divmod(core, 2)
        out[b, hh * HSH : (hh + 1) * HSH, :] = results[core]["out"]
    return out


def kernel(img_stack: np.ndarray, filts: np.ndarray) -> np.ndarray:
    nc = get_nc()
    in_maps = prepare_in_maps(img_stack, filts)
    res = run_bass_kernel_spmd(nc, in_maps, list(range(N_CORES)))
    return assemble_out(res.results)
